# revision 4
# baseline (speedup 1.0000x reference)
"""BCE + weighted Dice loss on 8 Trainium2 NeuronCores (fp8 edition).

Full inputs logits/targets [4,3,128,128,128] f32 are sharded along the depth
axis D=128 into 8 slices of 16 and converted to fp8-e4m3 on the host: targets
are {0,1} so exact; logits rounding (~3.6% rms relative, zero-mean) washes out
over 25M-element reductions and biases the loss by only ~1e-4 relative, far
inside the 2e-2 gate.  fp8 halves DMA vs bf16 (6.3 MB/core total) and lets
every matmul run double-pumped (2 cols/cycle) on the PE.

Math (s := sigmoid(-x)):
  sum(prob)    = N - sum(s)             prob = sigmoid(x)
  sum(prob*t)  = sum(t) - sum(s*t)
  bce_sum      = -sum(ln s) - sum(x*t)
  pred         = (x >= 0.5)

Engine split (per core, 3 "quads" of 4 (b,c) slabs, [128, 8192] fp8 tiles):
  ScalarE (critical path, ~46us busy):
      sigmoid(-x) per quad -> s8 fp8 (+accum sum s), then one table switch,
      ln(s8 + 1e-5) per quad (+accum sum ln s; the 1e-5 bias guards the
      fp8 underflow s8==0 for x>6.9 which would give -inf).
  TensorE (all fp8 double-pumped, ~43us):
      diag-trick matmuls: sum(s*t) global, sum(x*t) global, sum(t*pred)
      per slab; ones-row matmuls: sum(t), sum(pred) per slab.
  VectorE (~25us): pred = (x>=0.5) fp8 2x mode, PSUM diag extractions +
      reduce-to-stats, row-bank copies.

The diagonal trick: accumulating chunk matmuls A[:,c128].T @ B[:,c128] into
one PSUM bank leaves sum_c sum_p A[p,cm]*B[p,cn] at [m,n]; the diagonal
m == n carries sum(A*B).  Masking by the identity (tiny fp8 input) and a
TS-reduce recovers the scalar without slow DVE reductions.

Device outputs per core:
  stats [128, 24] f32:
      cols 0-3  sigmoid accums (quad0 split in two + quads 1,2)
      cols 4-6  ln accums per quad
      col  7    sum(x*t) global (diag extract+reduce)
      col  8    sum(s*t) global (diag extract+reduce)
      cols 9-20 sum(t*pred) per slab s=0..11 (diag extract+reduce)
  rows [128, 2048] f32: copies of the 4 row PSUM banks (t banks 0-1 at
      cols 0-1023, pred banks at 1024-2047); slab s lives in bank s//6 at
      partition ((s%6)%3)*32, cols ((s%6)//3)*256 .. +256.
"""

import sys

if "/opt/trn_rl_repo" not in sys.path:
    sys.path.insert(0, "/opt/trn_rl_repo")

import numpy as np

import concourse.bacc as bacc
import concourse.mybir as mybir
from concourse import tile
from concourse.alu_op_type import AluOpType
from concourse.bass_utils import run_bass_kernel_spmd

# Problem geometry (hardcoded per harness contract).
B, C, D, H, W = 4, 3, 128, 128, 128
N_CORES = 8
D_SHARD = D // N_CORES            # 16
SLABS = B * C                     # 12 (b,c) slabs per core
P = 128
F = D_SHARD * H * W // P          # 2048 free elems per slab per partition
N_TOTAL = B * C * D * H * W
QUADS = 3
QS = SLABS // QUADS               # 4 slabs per quad
QF = QS * F                       # 8192

_CACHED = {}


def _build():
    if "nc" in _CACHED:
        return _CACHED["nc"]
    AFT = mybir.ActivationFunctionType
    f32 = mybir.dt.float32
    fp8 = mybir.dt.float8e4

    nc = bacc.Bacc("TRN2", target_bir_lowering=False, debug=False,
                   num_devices=N_CORES)
    x_d = nc.dram_tensor("logits", [QUADS, P, QF], fp8, kind="ExternalInput")
    t_d = nc.dram_tensor("targets", [QUADS, P, QF], fp8, kind="ExternalInput")
    id_d = nc.dram_tensor("ident", [P, 128], fp8, kind="ExternalInput")
    st_d = nc.dram_tensor("stats", [P, 24], f32, kind="ExternalOutput")
    rw_d = nc.dram_tensor("rows", [P, 2048], f32, kind="ExternalOutput")

    with tile.TileContext(nc) as tc:
        with (
            tc.tile_pool(name="data", bufs=1) as data_pool,
            tc.tile_pool(name="misc", bufs=1) as misc_pool,
            tc.tile_pool(name="psum", bufs=1, space="PSUM") as psum_pool,
        ):
            stats = misc_pool.tile([P, 24], f32)
            nc.vector.memset(stats[:], 0.0)
            ones = misc_pool.tile([P, 1], fp8)
            nc.vector.memset(ones[:], 1.0)
            lnbias = misc_pool.tile([P, 1], f32)
            nc.vector.memset(lnbias[:], 1e-5)
            ident = misc_pool.tile([P, 128], fp8)
            nc.sync.dma_start(ident[:], id_d[:])

            # All quad tiles live for the whole kernel (SBUF ~13 MB).
            xq = [data_pool.tile([P, QF], fp8, name=f"xq{q}") for q in range(QUADS)]
            tq = [data_pool.tile([P, QF], fp8, name=f"tq{q}") for q in range(QUADS)]
            sq = [data_pool.tile([P, QF], fp8, name=f"sq{q}") for q in range(QUADS)]
            pq = [data_pool.tile([P, QF], fp8, name=f"pq{q}") for q in range(QUADS)]
            lnout = misc_pool.tile([P, QF], fp8)

            # Input DMA: x first (ScalarE is the critical consumer), quad 0
            # split so the first sigmoid can start early; t on another queue.
            for k in range(2):
                sl = slice(k * QF // 2, (k + 1) * QF // 2)
                nc.sync.dma_start(xq[0][:, sl], x_d[0][:, sl])
            nc.sync.dma_start(xq[1][:], x_d[1])
            nc.sync.dma_start(xq[2][:], x_d[2])
            for q in range(QUADS):
                nc.gpsimd.dma_start(tq[q][:], t_d[q])

            # PSUM banks (8): st, xt, 2 rotating tp, 2+2 row banks.
            p_st = psum_pool.tile([P, 128], f32, name="p_st", tag="p_st")
            p_xt = psum_pool.tile([P, 128], f32, name="p_xt", tag="p_xt")
            p_tp = [psum_pool.tile([P, 128], f32, name=f"p_tp{i}", tag=f"p_tp{i}")
                    for i in range(2)]
            p_tr = [psum_pool.tile([P, 512], f32, name=f"p_tr{i}", tag=f"p_tr{i}")
                    for i in range(2)]
            p_pr = [psum_pool.tile([P, 512], f32, name=f"p_pr{i}", tag=f"p_pr{i}")
                    for i in range(2)]

            # ---- ScalarE chain: all sigmoids, then all lns (2 table loads).
            for k in range(2):
                sl = slice(k * QF // 2, (k + 1) * QF // 2)
                nc.scalar.activation(sq[0][:, sl], xq[0][:, sl], AFT.Sigmoid,
                                     scale=-1.0, accum_out=stats[:, k:k + 1])
            nc.scalar.activation(sq[1][:], xq[1][:], AFT.Sigmoid, scale=-1.0,
                                 accum_out=stats[:, 2:3])
            nc.scalar.activation(sq[2][:], xq[2][:], AFT.Sigmoid, scale=-1.0,
                                 accum_out=stats[:, 3:4])
            for q in range(QUADS):
                nc.scalar.activation(lnout[:], sq[q][:], AFT.Ln,
                                     bias=lnbias[:, 0:1],
                                     accum_out=stats[:, 4 + q:5 + q])

            # ---- VectorE: pred per quad (fp8 2x mode).
            for q in range(QUADS):
                nc.vector.tensor_scalar(out=pq[q][:], in0=xq[q][:],
                                        scalar1=0.5, scalar2=None,
                                        op0=AluOpType.is_ge)

            # ---- TensorE per quad, in dependency order:
            #   t-rows (t), xt diag (x,t), pred-rows (pred), tp diag
            #   (pred,t), st diag (s,t).
            for q in range(QUADS):
                for j in range(QS):
                    s_i = q * QS + j
                    base = j * F
                    k6 = s_i % 6
                    row = (k6 % 3) * 32
                    colblk = (k6 // 3) * 256
                    tr_bank = p_tr[s_i // 6]
                    for c in range(8):
                        sl = slice(base + c * 256, base + (c + 1) * 256)
                        nc.tensor.matmul(
                            tr_bank[row:row + 1, colblk:colblk + 256],
                            ones[:], tq[q][:, sl], start=(c == 0), stop=(c == 7))
                for j in range(QS):
                    s_i = q * QS + j
                    base = j * F
                    first = s_i == 0
                    last = s_i == SLABS - 1
                    for c in range(16):
                        sl = slice(base + c * 128, base + (c + 1) * 128)
                        nc.tensor.matmul(p_xt[:, :], xq[q][:, sl], tq[q][:, sl],
                                         start=(first and c == 0),
                                         stop=(last and c == 15))
                for j in range(QS):
                    s_i = q * QS + j
                    base = j * F
                    k6 = s_i % 6
                    row = (k6 % 3) * 32
                    colblk = (k6 // 3) * 256
                    pr_bank = p_pr[s_i // 6]
                    for c in range(8):
                        sl = slice(base + c * 256, base + (c + 1) * 256)
                        nc.tensor.matmul(
                            pr_bank[row:row + 1, colblk:colblk + 256],
                            ones[:], pq[q][:, sl], start=(c == 0), stop=(c == 7))
                for j in range(QS):
                    s_i = q * QS + j
                    base = j * F
                    tp_bank = p_tp[s_i % 2]
                    for c in range(16):
                        sl = slice(base + c * 128, base + (c + 1) * 128)
                        nc.tensor.matmul(tp_bank[:, :], pq[q][:, sl],
                                         tq[q][:, sl],
                                         start=(c == 0), stop=(c == 15))
                    # Extract this slab's diag on DVE and fold into stats.
                    mtp = misc_pool.tile([P, 128], f32, name=f"mtp{s_i}",
                                         tag="mtp", bufs=2)
                    nc.vector.tensor_tensor(out=mtp[:], in0=tp_bank[:, :],
                                            in1=ident[:], op=AluOpType.mult)
                    nc.vector.tensor_scalar(out=mtp[:], in0=mtp[:],
                                            scalar1=1.0, scalar2=0.0,
                                            op0=AluOpType.mult,
                                            op1=AluOpType.add,
                                            accum_out=stats[:, 9 + s_i:10 + s_i])
                for j in range(QS):
                    s_i = q * QS + j
                    base = j * F
                    first = s_i == 0
                    last = s_i == SLABS - 1
                    for c in range(16):
                        sl = slice(base + c * 128, base + (c + 1) * 128)
                        nc.tensor.matmul(p_st[:, :], sq[q][:, sl], tq[q][:, sl],
                                         start=(first and c == 0),
                                         stop=(last and c == 15))

            # ---- Epilogue: global diag extractions + row-bank copy-out.
            mxt = misc_pool.tile([P, 128], f32)
            nc.vector.tensor_tensor(out=mxt[:], in0=p_xt[:, :], in1=ident[:],
                                    op=AluOpType.mult)
            nc.vector.tensor_scalar(out=mxt[:], in0=mxt[:], scalar1=1.0,
                                    scalar2=0.0, op0=AluOpType.mult,
                                    op1=AluOpType.add,
                                    accum_out=stats[:, 7:8])
            mst = misc_pool.tile([P, 128], f32)
            nc.vector.tensor_tensor(out=mst[:], in0=p_st[:, :], in1=ident[:],
                                    op=AluOpType.mult)
            nc.vector.tensor_scalar(out=mst[:], in0=mst[:], scalar1=1.0,
                                    scalar2=0.0, op0=AluOpType.mult,
                                    op1=AluOpType.add,
                                    accum_out=stats[:, 8:9])

            rows = misc_pool.tile([P, 2048], f32)
            for i in range(2):
                nc.vector.tensor_copy(rows[:, 512 * i:512 * (i + 1)],
                                      p_tr[i][:, :])
                nc.vector.tensor_copy(rows[:, 1024 + 512 * i:1024 + 512 * (i + 1)],
                                      p_pr[i][:, :])
            nc.sync.dma_start(rw_d[:], rows[:])
            nc.sync.dma_start(st_d[:], stats[:])

    nc.compile()
    _CACHED["nc"] = nc
    return nc


def _shard_inputs(logits: np.ndarray, targets: np.ndarray):
    import ml_dtypes

    f8 = ml_dtypes.float8_e4m3
    xb = np.ascontiguousarray(logits, dtype=np.float32).astype(f8)
    tb = np.ascontiguousarray(targets, dtype=np.float32).astype(f8)
    eye = np.eye(P, 128, dtype=np.float32).astype(f8)
    in_maps = []
    for i in range(N_CORES):
        sl = slice(i * D_SHARD, (i + 1) * D_SHARD)
        x = np.ascontiguousarray(xb[:, :, sl]).reshape(QUADS, P, QF)
        t = np.ascontiguousarray(tb[:, :, sl]).reshape(QUADS, P, QF)
        in_maps.append({"logits": x, "targets": t, "ident": eye})
    return in_maps


def _combine(results):
    """Host-side reduction of per-core partials to the scalar loss."""
    EPS = 1e-9
    S_s = 0.0
    S_l = 0.0
    S_xt = 0.0
    S_st = 0.0
    S_tp = np.zeros(SLABS)
    S_t = np.zeros(SLABS)
    S_pred = np.zeros(SLABS)
    for r in results:
        st = r["stats"].astype(np.float64)
        S_s += st[:, 0:4].sum()
        S_l += st[:, 4:7].sum()
        S_xt += st[:, 7].sum()
        S_st += st[:, 8].sum()
        S_tp += st[:, 9:21].sum(axis=0)
        rw = r["rows"].astype(np.float64)
        for s_i in range(SLABS):
            k6 = s_i % 6
            row = (k6 % 3) * 32
            col = (k6 // 3) * 256
            bank = s_i // 6
            S_t[s_i] += rw[row, 512 * bank + col:512 * bank + col + 256].sum()
            S_pred[s_i] += rw[row, 1024 + 512 * bank + col:
                              1024 + 512 * bank + col + 256].sum()

    sum_prob = N_TOTAL - S_s
    sum_pt = S_t.sum() - S_st                 # sum(prob * t)
    sum_sp = -S_l                             # sum(softplus(x))
    bce = (sum_sp - S_xt) / N_TOTAL

    union = sum_prob + S_t.sum()
    inter = 2.0 * sum_pt
    dice_loss = 1.0 - (inter + EPS) / union

    score = np.where(
        (S_t == 0) & (S_pred == 0),
        np.ones_like(S_t),
        (2.0 * S_tp + EPS) / (S_t + S_pred),
    ).reshape(B, C)
    per_class = score.mean(axis=0)

    loss = (bce + dice_loss * 0.5 + per_class[0] * 0.2
            + per_class[1] * 0.1 + per_class[2] * 0.2)
    return np.float32(loss)


def kernel(logits: np.ndarray, targets: np.ndarray) -> np.ndarray:
    nc = _build()
    in_maps = _shard_inputs(np.asarray(logits), np.asarray(targets))
    res = run_bass_kernel_spmd(nc, in_maps, list(range(N_CORES)))
    return _combine(res.results)


# revision 7
# speedup vs baseline: 1.1810x; 1.1810x over previous
"""BCE + weighted Dice loss on 8 Trainium2 NeuronCores (fp8, v4).

Full inputs logits/targets [4,3,128,128,128] f32 are sharded along depth
D=128 into 8 slices of 16, converted to fp8-e4m3 on the host (targets {0,1}
exact; logits 3.6% rms rounding washes out over 25M-element sums, biasing
the loss ~1e-4 relative — far inside the 2e-2 gate), and packed in an
AUGMENTED layout: each 128-column chunk carries 4 extra columns
[1, 0, 0, 0].  A diag-trick matmul whose rhs is an augmented chunk then
produces, in PSUM column 128, the column-sums of its lhsT operand for free:

  xt bank (lhsT=t, rhs=x_aug):   diag = x*t,     col128 = sum(t)   per slab
  tp bank (lhsT=pred, rhs=t_aug): diag = t*pred,  col128 = sum(pred) per slab
  st bank (lhsT=s, rhs=t_aug):    diag = s*t (global)

which eliminates all ones-row matmuls.  All PE operands are fp8 -> double
pumped (2 cols/cycle).  ScalarE runs dense over the augmented tiles (the
deterministic contribution of the [1,0,0,0] columns to the sigmoid/ln
accumulators is subtracted exactly on the host).

Math (s := sigmoid(-x)):
  sum(prob) = N - sum(s);  sum(prob*t) = sum(t) - sum(s*t)
  bce_sum   = -sum(ln s) - sum(x*t);   pred = (x >= 0.5)
  ln uses bias 1e-5: guards ln(0) when fp8 underflows s for x > 6.9.

Input DMA rides a SINGLE queue: the DMA engines round-robin fairly across
outstanding transfers, so one queue = strict arrival order at ~400 GB/s,
while multiple queues delay the critical first chunks.  Order: x slabs
first (ScalarE is the critical path), t interleaved just-in-time for PE.

Engine budget per core: ScalarE ~46us (2 activation passes, 2 table loads)
= critical path; TensorE ~34us (3 fp8 diag quantities); VectorE ~31us
(pred + per-slab PSUM extractions); DMA in 6.5MB ~16us.

Device outputs per core, one stats tile [128, 64] f32 (+ small late tile):
  cols 0-4   sigmoid accums (5 instrs: slabs [0],[1],[2-3],[4-7],[8-11])
  cols 5-7   ln accums ([0-3],[4-7],[8-11])  (in stats2 [128,3])
  col  8     sum(s*t) global (masked diag reduce)
  cols 9-20  sum(t*pred) per slab      cols 21-32 sum(x*t) per slab
  cols 33-44 sum(pred) per slab        cols 45-56 sum(t) per slab
"""

import sys

if "/opt/trn_rl_repo" not in sys.path:
    sys.path.insert(0, "/opt/trn_rl_repo")

import numpy as np

import concourse.bacc as bacc
import concourse.mybir as mybir
from concourse import tile
from concourse.alu_op_type import AluOpType
from concourse.bass_utils import run_bass_kernel_spmd

# Problem geometry (hardcoded per harness contract).
B, C, D, H, W = 4, 3, 128, 128, 128
N_CORES = 8
D_SHARD = D // N_CORES            # 16
SLABS = B * C                     # 12 (b,c) slabs per core
P = 128
F = D_SHARD * H * W // P          # 2048 real cols per slab
N_TOTAL = B * C * D * H * W
NCH = F // 128                    # 16 chunks per slab
AUG = 132                         # 128 real + [1,0,0,0]
SF = NCH * AUG                    # 2112 aug cols per slab
TF = SLABS * SF                   # 25344 aug cols total

_CACHED = {}


def _build():
    if "nc" in _CACHED:
        return _CACHED["nc"]
    AFT = mybir.ActivationFunctionType
    f32 = mybir.dt.float32
    fp8 = mybir.dt.float8e4

    nc = bacc.Bacc("TRN2", target_bir_lowering=False, debug=False,
                   num_devices=N_CORES)
    x_d = nc.dram_tensor("logits", [SLABS, P, SF], fp8, kind="ExternalInput")
    t_d = nc.dram_tensor("targets", [SLABS, P, SF], fp8, kind="ExternalInput")
    id_d = nc.dram_tensor("ident", [P, AUG], fp8, kind="ExternalInput")
    st_d = nc.dram_tensor("stats", [P, 64], f32, kind="ExternalOutput")
    s2_d = nc.dram_tensor("stats2", [P, 3], f32, kind="ExternalOutput")

    SIG_SPLIT = [(0, 1), (1, 2), (2, 4), (4, 8), (8, 12)]
    LN_SPLIT = [(0, 4), (4, 8), (8, 12)]
    XCH = [(0, 1), (1, 2), (2, 4), (4, 6), (6, 8), (8, 10), (10, 12)]
    TCH = [(0, 3), (3, 6), (6, 9), (9, 12)]
    # Single-queue arrival order: x leads, t just-in-time.
    ORDER = [("x", 0), ("x", 1), ("x", 2), ("t", 0), ("x", 3), ("x", 4),
             ("t", 1), ("x", 5), ("x", 6), ("t", 2), ("t", 3)]

    with tile.TileContext(nc) as tc:
        with (
            tc.tile_pool(name="data", bufs=1) as data_pool,
            tc.tile_pool(name="misc", bufs=1) as misc_pool,
            tc.tile_pool(name="psum", bufs=1, space="PSUM") as psum_pool,
        ):
            stats = misc_pool.tile([P, 64], f32)
            nc.vector.memset(stats[:], 0.0)
            stats2 = misc_pool.tile([P, 3], f32)
            nc.vector.memset(stats2[:], 0.0)
            lnbias = misc_pool.tile([P, 1], f32)
            nc.vector.memset(lnbias[:], 1e-5)
            ident = misc_pool.tile([P, AUG], fp8)

            xa = data_pool.tile([P, TF], fp8, name="xa")
            ta = data_pool.tile([P, TF], fp8, name="ta")
            s8 = data_pool.tile([P, TF], fp8, name="s8")
            pr = data_pool.tile([P, TF], fp8, name="pr")
            lo = data_pool.tile([P, TF], fp8, name="lo")

            # ---- Input DMA: one queue, strict order.
            first = True
            for kind, i in ORDER:
                if kind == "x":
                    a, b = XCH[i]
                    nc.sync.dma_start(xa[:, a * SF:b * SF], x_d[a:b])
                    if first:
                        nc.sync.dma_start(ident[:], id_d[:])
                        first = False
                else:
                    a, b = TCH[i]
                    nc.sync.dma_start(ta[:, a * SF:b * SF], t_d[a:b])

            # ---- ScalarE: sigmoid chain then ln chain (2 table loads).
            for k, (a, b) in enumerate(SIG_SPLIT):
                nc.scalar.activation(s8[:, a * SF:b * SF], xa[:, a * SF:b * SF],
                                     AFT.Sigmoid, scale=-1.0,
                                     accum_out=stats[:, k:k + 1])
            for k, (a, b) in enumerate(LN_SPLIT):
                nc.scalar.activation(lo[:, a * SF:b * SF],
                                     s8[:, a * SF:b * SF], AFT.Ln,
                                     bias=lnbias[:, 0:1],
                                     accum_out=stats2[:, k:k + 1])

            # ---- VectorE: pred (dense over aug tiles, fp8 2x).
            for a, b in [(0, 2), (2, 4), (4, 6), (6, 8), (8, 10), (10, 12)]:
                nc.vector.tensor_scalar(out=pr[:, a * SF:b * SF],
                                        in0=xa[:, a * SF:b * SF],
                                        scalar1=0.5, scalar2=None,
                                        op0=AluOpType.is_ge)

            # ---- PSUM banks: st global + rotating xt/tp.
            p_st = psum_pool.tile([P, AUG], f32, name="p_st", tag="p_st")
            p_xt = [psum_pool.tile([P, AUG], f32, name=f"p_xt{i}", tag=f"p_xt{i}")
                    for i in range(2)]
            p_tp = [psum_pool.tile([P, AUG], f32, name=f"p_tp{i}", tag=f"p_tp{i}")
                    for i in range(2)]

            # ---- TensorE + extractions per slab.
            for s in range(SLABS):
                base = s * SF
                xt_b = p_xt[s % 2]
                tp_b = p_tp[s % 2]
                for c in range(NCH):
                    off = base + c * AUG
                    nc.tensor.matmul(xt_b[:, :], ta[:, off:off + 128],
                                     xa[:, off:off + AUG],
                                     start=(c == 0), stop=(c == NCH - 1))
                for c in range(NCH):
                    off = base + c * AUG
                    nc.tensor.matmul(tp_b[:, :], pr[:, off:off + 128],
                                     ta[:, off:off + AUG],
                                     start=(c == 0), stop=(c == NCH - 1))
                for c in range(NCH):
                    off = base + c * AUG
                    nc.tensor.matmul(p_st[:, :], s8[:, off:off + 128],
                                     ta[:, off:off + AUG],
                                     start=(s == 0 and c == 0),
                                     stop=(s == SLABS - 1 and c == NCH - 1))

                mx = misc_pool.tile([P, AUG], f32, name=f"mx{s}", tag="mx",
                                    bufs=2)
                nc.vector.tensor_tensor(out=mx[:], in0=xt_b[:, :],
                                        in1=ident[:], op=AluOpType.mult)
                nc.vector.tensor_scalar(out=mx[:], in0=mx[:], scalar1=1.0,
                                        scalar2=0.0, op0=AluOpType.mult,
                                        op1=AluOpType.add,
                                        accum_out=stats[:, 21 + s:22 + s])
                nc.vector.tensor_copy(stats[:, 45 + s:46 + s],
                                      xt_b[:, 128:129])
                mt = misc_pool.tile([P, AUG], f32, name=f"mt{s}", tag="mt",
                                    bufs=2)
                nc.vector.tensor_tensor(out=mt[:], in0=tp_b[:, :],
                                        in1=ident[:], op=AluOpType.mult)
                nc.vector.tensor_scalar(out=mt[:], in0=mt[:], scalar1=1.0,
                                        scalar2=0.0, op0=AluOpType.mult,
                                        op1=AluOpType.add,
                                        accum_out=stats[:, 9 + s:10 + s])
                nc.vector.tensor_copy(stats[:, 33 + s:34 + s],
                                      tp_b[:, 128:129])

            # ---- st global extraction.
            ms = misc_pool.tile([P, AUG], f32)
            nc.vector.tensor_tensor(out=ms[:], in0=p_st[:, :], in1=ident[:],
                                    op=AluOpType.mult)
            nc.vector.tensor_scalar(out=ms[:], in0=ms[:], scalar1=1.0,
                                    scalar2=0.0, op0=AluOpType.mult,
                                    op1=AluOpType.add,
                                    accum_out=stats[:, 8:9])
            nc.sync.dma_start(st_d[:], stats[:])
            nc.sync.dma_start(s2_d[:], stats2[:])

    nc.compile()
    _CACHED["nc"] = nc
    return nc


def _pack_aug(a):
    """[12, P, F] fp8 -> [12, P, SF] with [1,0,0,0] after each 128 cols."""
    import ml_dtypes

    f8 = ml_dtypes.float8_e4m3
    n = a.reshape(SLABS, P, NCH, 128)
    out = np.zeros((SLABS, P, NCH, AUG), dtype=f8)
    out[..., :128] = n
    out[..., 128] = f8(1.0)
    return out.reshape(SLABS, P, SF)


def _shard_inputs(logits: np.ndarray, targets: np.ndarray):
    import ml_dtypes

    f8 = ml_dtypes.float8_e4m3
    xb = np.ascontiguousarray(logits, dtype=np.float32).astype(f8)
    tb = np.ascontiguousarray(targets, dtype=np.float32).astype(f8)
    eye = np.zeros((P, AUG), dtype=np.float32)
    eye[:, :128] = np.eye(P, 128, dtype=np.float32)
    eye = eye.astype(f8)
    in_maps = []
    for i in range(N_CORES):
        sl = slice(i * D_SHARD, (i + 1) * D_SHARD)
        x = np.ascontiguousarray(xb[:, :, sl]).reshape(SLABS, P, F)
        t = np.ascontiguousarray(tb[:, :, sl]).reshape(SLABS, P, F)
        in_maps.append({"logits": _pack_aug(x), "targets": _pack_aug(t),
                        "ident": eye})
    return in_maps


def _aug_constants():
    """Exact per-core contribution of the [1,0,0,0] aug columns to the
    sigmoid and ln accumulators (deterministic; table assumed ~= math)."""
    import ml_dtypes

    f8 = ml_dtypes.float8_e4m3
    n_aug = SLABS * NCH * P          # cells per aug column position
    sig1 = 1.0 / (1.0 + np.exp(1.0))          # sigmoid(-1)
    sig0 = 0.5                                # sigmoid(-0)
    k_sig = n_aug * (sig1 + 3 * sig0)
    s1_8 = float(np.float32(sig1).astype(f8))  # fp8 round of sigmoid(-1)
    k_ln = n_aug * (np.log(s1_8 + 1e-5) + 3 * np.log(0.5 + 1e-5))
    return k_sig, k_ln


def _combine(results):
    """Host-side reduction of per-core partials to the scalar loss."""
    EPS = 1e-9
    k_sig, k_ln = _aug_constants()
    S_s = 0.0
    S_l = 0.0
    S_xt = 0.0
    S_st = 0.0
    S_tp = np.zeros(SLABS)
    S_t = np.zeros(SLABS)
    S_pred = np.zeros(SLABS)
    for r in results:
        st = r["stats"].astype(np.float64)
        s2 = r["stats2"].astype(np.float64)
        S_s += st[:, 0:5].sum() - k_sig
        S_l += s2.sum() - k_ln
        S_st += st[:, 8].sum()
        S_tp += st[:, 9:21].sum(axis=0)
        S_xt += st[:, 21:33].sum()
        S_pred += st[:, 33:45].sum(axis=0)
        S_t += st[:, 45:57].sum(axis=0)

    sum_prob = N_TOTAL - S_s
    sum_pt = S_t.sum() - S_st                 # sum(prob * t)
    sum_sp = -S_l                             # sum(softplus(x))
    bce = (sum_sp - S_xt) / N_TOTAL

    union = sum_prob + S_t.sum()
    inter = 2.0 * sum_pt
    dice_loss = 1.0 - (inter + EPS) / union

    score = np.where(
        (S_t == 0) & (S_pred == 0),
        np.ones_like(S_t),
        (2.0 * S_tp + EPS) / (S_t + S_pred),
    ).reshape(B, C)
    per_class = score.mean(axis=0)

    loss = (bce + dice_loss * 0.5 + per_class[0] * 0.2
            + per_class[1] * 0.1 + per_class[2] * 0.2)
    return np.float32(loss)


def kernel(logits: np.ndarray, targets: np.ndarray) -> np.ndarray:
    nc = _build()
    in_maps = _shard_inputs(np.asarray(logits), np.asarray(targets))
    res = run_bass_kernel_spmd(nc, in_maps, list(range(N_CORES)))
    return _combine(res.results)


# revision 15
# speedup vs baseline: 1.2594x; 1.0664x over previous
"""BCE + weighted Dice loss on 8 Trainium2 NeuronCores (fp8, v4).

Full inputs logits/targets [4,3,128,128,128] f32 are sharded along depth
D=128 into 8 slices of 16, converted to fp8-e4m3 on the host (targets {0,1}
exact; logits 3.6% rms rounding washes out over 25M-element sums, biasing
the loss ~1e-4 relative — far inside the 2e-2 gate), and packed in an
AUGMENTED layout: each 128-column chunk carries 4 extra columns
[1, 0, 0, 0].  A diag-trick matmul whose rhs is an augmented chunk then
produces, in PSUM column 128, the column-sums of its lhsT operand for free:

  xt bank (lhsT=t, rhs=x_aug):   diag = x*t,     col128 = sum(t)   per slab
  tp bank (lhsT=pred, rhs=t_aug): diag = t*pred,  col128 = sum(pred) per slab
  st bank (lhsT=s, rhs=t_aug):    diag = s*t (global)

which eliminates all ones-row matmuls.  All PE operands are fp8 -> double
pumped (2 cols/cycle).  ScalarE runs dense over the augmented tiles (the
deterministic contribution of the [1,0,0,0] columns to the sigmoid/ln
accumulators is subtracted exactly on the host).

Math (s := sigmoid(-x)):
  sum(prob) = N - sum(s);  sum(prob*t) = sum(t) - sum(s*t)
  bce_sum   = -sum(ln s) - sum(x*t);   pred = (x >= 0.5)
  ln uses bias 1e-5: guards ln(0) when fp8 underflows s for x > 6.9.

Input DMA rides a SINGLE queue: the DMA engines round-robin fairly across
outstanding transfers, so one queue = strict arrival order at ~400 GB/s,
while multiple queues delay the critical first chunks.  Order: x slabs
first (ScalarE is the critical path), t interleaved just-in-time for PE.

Engine budget per core: ScalarE ~46us (2 activation passes, 2 table loads)
= critical path; TensorE ~34us (3 fp8 diag quantities); VectorE ~31us
(pred + per-slab PSUM extractions); DMA in 6.5MB ~16us.

Device outputs per core, one stats tile [128, 64] f32 (+ small late tile):
  cols 0-4   sigmoid accums (5 instrs: slabs [0],[1],[2-3],[4-7],[8-11])
  cols 5-7   ln accums ([0-3],[4-7],[8-11])  (in stats2 [128,3])
  col  8     sum(s*t) global (masked diag reduce)
  cols 9-20  sum(t*pred) per slab      cols 21-32 sum(x*t) per slab
  cols 33-44 sum(pred) per slab        cols 45-56 sum(t) per slab
"""

import sys

if "/opt/trn_rl_repo" not in sys.path:
    sys.path.insert(0, "/opt/trn_rl_repo")

import numpy as np

import concourse.bacc as bacc
import concourse.mybir as mybir
from concourse import tile
from concourse.alu_op_type import AluOpType
from concourse.bass_utils import run_bass_kernel_spmd

# Problem geometry (hardcoded per harness contract).
B, C, D, H, W = 4, 3, 128, 128, 128
N_CORES = 8
D_SHARD = D // N_CORES            # 16
SLABS = B * C                     # 12 (b,c) slabs per core
P = 128
F = D_SHARD * H * W // P          # 2048 real cols per slab
N_TOTAL = B * C * D * H * W
NCH = F // 128                    # 16 chunks per slab
AUG = 132                         # 128 real + [1,0,0,0]
SF = NCH * AUG                    # 2112 aug cols per slab
TF = SLABS * SF                   # 25344 aug cols total

_CACHED = {}


def _build():
    if "nc" in _CACHED:
        return _CACHED["nc"]
    AFT = mybir.ActivationFunctionType
    f32 = mybir.dt.float32
    fp8 = mybir.dt.float8e4

    nc = bacc.Bacc("TRN2", target_bir_lowering=False, debug=False,
                   num_devices=N_CORES)
    x_d = nc.dram_tensor("logits", [SLABS, P, SF], fp8, kind="ExternalInput")
    t_d = nc.dram_tensor("targets", [SLABS, P, SF], fp8, kind="ExternalInput")
    id_d = nc.dram_tensor("ident", [P, AUG], fp8, kind="ExternalInput")
    st_d = nc.dram_tensor("stats", [P, 64], f32, kind="ExternalOutput")
    s2_d = nc.dram_tensor("stats2", [P, 3], f32, kind="ExternalOutput")

    SIG_SPLIT = [(0, 1), (1, 2), (2, 4), (4, 8), (8, 12)]
    XCH = [(0, 1), (1, 2), (2, 4), (4, 6), (6, 8), (8, 10), (10, 12)]
    TCH = [(0, 3), (3, 6), (6, 9), (9, 12)]
    # Single-queue arrival order: x leads, t just-in-time.
    ORDER = [("x", 0), ("x", 1), ("x", 2), ("t", 0), ("x", 3), ("x", 4),
             ("t", 1), ("x", 5), ("x", 6), ("t", 2), ("t", 3)]

    with tile.TileContext(nc) as tc:
        with (
            tc.tile_pool(name="data", bufs=1) as data_pool,
            tc.tile_pool(name="misc", bufs=1) as misc_pool,
            tc.tile_pool(name="psum", bufs=1, space="PSUM") as psum_pool,
        ):
            stats = misc_pool.tile([P, 64], f32)
            nc.vector.memset(stats[:], 0.0)
            stats2 = misc_pool.tile([P, 3], f32)
            nc.vector.memset(stats2[:], 0.0)
            lnbias = misc_pool.tile([P, 1], f32)
            nc.vector.memset(lnbias[:], 1e-5)
            ident = misc_pool.tile([P, AUG], fp8)

            NC = SLABS * NCH          # 192 chunks total
            xa = data_pool.tile([P, NC, AUG], fp8, name="xa")
            ta = data_pool.tile([P, NC, AUG], fp8, name="ta")
            s8 = data_pool.tile([P, NC, AUG], fp8, name="s8")
            pr = data_pool.tile([P, NC, AUG], fp8, name="pr")
            lo = data_pool.tile([P, NC, AUG], fp8, name="lo")

            # ---- Input DMA: one queue, strict order.
            first = True
            for kind, i in ORDER:
                if kind == "x":
                    a, b = XCH[i]
                    nc.sync.dma_start(xa[:, a * NCH:b * NCH, :], x_d[a:b])
                    if first:
                        nc.sync.dma_start(ident[:], id_d[:])
                        first = False
                else:
                    a, b = TCH[i]
                    nc.sync.dma_start(ta[:, a * NCH:b * NCH, :], t_d[a:b])

            # ---- ScalarE: dummy sigmoid first so the auto-inserted
            # ACT_TABLE_LOAD runs during the DMA wait; then the sigmoid
            # chain, one table switch, and a single ln pass.  Both real
            # passes use strided APs covering only the 128 real columns of
            # each 132-column chunk, so no aug-column corrections exist.
            dummy = misc_pool.tile([P, 1], fp8)
            nc.scalar.activation(dummy[:], lnbias[:, 0:1], AFT.Sigmoid)
            for a, b in SIG_SPLIT:
                nc.scalar.activation(s8[:, a * NCH:b * NCH, 0:128],
                                     xa[:, a * NCH:b * NCH, 0:128],
                                     AFT.Sigmoid, scale=-1.0)
            nc.scalar.activation(lo[:, :, 0:128], s8[:, :, 0:128], AFT.Ln,
                                 bias=lnbias[:, 0:1],
                                 accum_out=stats2[:, 0:1])

            # ---- VectorE: pred (dense over aug tiles, fp8 2x).
            for a, b in [(0, 2), (2, 4), (4, 6), (6, 8), (8, 10), (10, 12)]:
                nc.vector.tensor_scalar(out=pr[:, a * NCH:b * NCH, :],
                                        in0=xa[:, a * NCH:b * NCH, :],
                                        scalar1=0.5, scalar2=None,
                                        op0=AluOpType.is_ge)

            # ---- PSUM banks: st global + rotating xt/tp.
            p_st = psum_pool.tile([P, AUG], f32, name="p_st", tag="p_st")
            p_xt = [psum_pool.tile([P, AUG], f32, name=f"p_xt{i}", tag=f"p_xt{i}")
                    for i in range(2)]
            p_tp = [psum_pool.tile([P, AUG], f32, name=f"p_tp{i}", tag=f"p_tp{i}")
                    for i in range(2)]

            # ---- TensorE + extractions per slab.
            for s in range(SLABS):
                xt_b = p_xt[s % 2]
                tp_b = p_tp[s % 2]
                for c in range(NCH):
                    k = s * NCH + c
                    nc.tensor.matmul(xt_b[:, :], ta[:, k, 0:128],
                                     xa[:, k, :],
                                     start=(c == 0), stop=(c == NCH - 1))
                for c in range(NCH):
                    k = s * NCH + c
                    nc.tensor.matmul(tp_b[:, :], pr[:, k, 0:128],
                                     ta[:, k, :],
                                     start=(c == 0), stop=(c == NCH - 1))
                for c in range(NCH):
                    k = s * NCH + c
                    nc.tensor.matmul(p_st[:, :], s8[:, k, 0:128],
                                     ta[:, k, :],
                                     start=(s == 0 and c == 0),
                                     stop=(s == SLABS - 1 and c == NCH - 1))

                mx = misc_pool.tile([P, AUG], f32, name=f"mx{s}", tag="mx",
                                    bufs=2)
                nc.vector.tensor_tensor(out=mx[:], in0=xt_b[:, :],
                                        in1=ident[:], op=AluOpType.mult)
                nc.vector.tensor_scalar(out=mx[:], in0=mx[:], scalar1=1.0,
                                        scalar2=0.0, op0=AluOpType.mult,
                                        op1=AluOpType.add,
                                        accum_out=stats[:, 21 + s:22 + s])
                nc.vector.tensor_copy(stats[:, 45 + s:46 + s],
                                      xt_b[:, 128:129])
                mt = misc_pool.tile([P, AUG], f32, name=f"mt{s}", tag="mt",
                                    bufs=2)
                nc.vector.tensor_tensor(out=mt[:], in0=tp_b[:, :],
                                        in1=ident[:], op=AluOpType.mult)
                nc.vector.tensor_scalar(out=mt[:], in0=mt[:], scalar1=1.0,
                                        scalar2=0.0, op0=AluOpType.mult,
                                        op1=AluOpType.add,
                                        accum_out=stats[:, 9 + s:10 + s])
                nc.vector.tensor_copy(stats[:, 33 + s:34 + s],
                                      tp_b[:, 128:129])

            # ---- st global extraction (+ sum(s) from its ones-column).
            ms = misc_pool.tile([P, AUG], f32)
            nc.vector.tensor_tensor(out=ms[:], in0=p_st[:, :], in1=ident[:],
                                    op=AluOpType.mult)
            nc.vector.tensor_scalar(out=ms[:], in0=ms[:], scalar1=1.0,
                                    scalar2=0.0, op0=AluOpType.mult,
                                    op1=AluOpType.add,
                                    accum_out=stats[:, 8:9])
            nc.vector.tensor_copy(stats[:, 0:1], p_st[:, 128:129])
            nc.sync.dma_start(st_d[:], stats[:])
            nc.sync.dma_start(s2_d[:], stats2[:])

    nc.compile()
    _CACHED["nc"] = nc
    return nc


def _pack_aug(a):
    """[12, P, F] fp8 -> [12, P, SF] with [1,0,0,0] after each 128 cols."""
    import ml_dtypes

    f8 = ml_dtypes.float8_e4m3
    n = a.reshape(SLABS, P, NCH, 128)
    out = np.zeros((SLABS, P, NCH, AUG), dtype=f8)
    out[..., :128] = n
    out[..., 128] = f8(1.0)
    return out.reshape(SLABS, P, SF)


def _shard_inputs(logits: np.ndarray, targets: np.ndarray):
    import ml_dtypes

    f8 = ml_dtypes.float8_e4m3
    xb = np.ascontiguousarray(logits, dtype=np.float32).astype(f8)
    tb = np.ascontiguousarray(targets, dtype=np.float32).astype(f8)
    eye = np.zeros((P, AUG), dtype=np.float32)
    eye[:, :128] = np.eye(P, 128, dtype=np.float32)
    eye = eye.astype(f8)
    in_maps = []
    for i in range(N_CORES):
        sl = slice(i * D_SHARD, (i + 1) * D_SHARD)
        x = np.ascontiguousarray(xb[:, :, sl]).reshape(SLABS, P, F)
        t = np.ascontiguousarray(tb[:, :, sl]).reshape(SLABS, P, F)
        in_maps.append({"logits": _pack_aug(x), "targets": _pack_aug(t),
                        "ident": eye})
    return in_maps


def _combine(results):
    """Host-side reduction of per-core partials to the scalar loss."""
    EPS = 1e-9
    S_s = 0.0
    S_l = 0.0
    S_xt = 0.0
    S_st = 0.0
    S_tp = np.zeros(SLABS)
    S_t = np.zeros(SLABS)
    S_pred = np.zeros(SLABS)
    for r in results:
        st = r["stats"].astype(np.float64)
        s2 = r["stats2"].astype(np.float64)
        S_s += st[:, 0].sum()
        S_l += s2[:, 0].sum()
        S_st += st[:, 8].sum()
        S_tp += st[:, 9:21].sum(axis=0)
        S_xt += st[:, 21:33].sum()
        S_pred += st[:, 33:45].sum(axis=0)
        S_t += st[:, 45:57].sum(axis=0)

    sum_prob = N_TOTAL - S_s
    sum_pt = S_t.sum() - S_st                 # sum(prob * t)
    sum_sp = -S_l                             # sum(softplus(x))
    bce = (sum_sp - S_xt) / N_TOTAL

    union = sum_prob + S_t.sum()
    inter = 2.0 * sum_pt
    dice_loss = 1.0 - (inter + EPS) / union

    score = np.where(
        (S_t == 0) & (S_pred == 0),
        np.ones_like(S_t),
        (2.0 * S_tp + EPS) / (S_t + S_pred),
    ).reshape(B, C)
    per_class = score.mean(axis=0)

    loss = (bce + dice_loss * 0.5 + per_class[0] * 0.2
            + per_class[1] * 0.1 + per_class[2] * 0.2)
    return np.float32(loss)


def kernel(logits: np.ndarray, targets: np.ndarray) -> np.ndarray:
    nc = _build()
    in_maps = _shard_inputs(np.asarray(logits), np.asarray(targets))
    res = run_bass_kernel_spmd(nc, in_maps, list(range(N_CORES)))
    return _combine(res.results)


# revision 18
# speedup vs baseline: 1.2637x; 1.0034x over previous
"""BCE + weighted Dice loss on 8 Trainium2 NeuronCores (fp8, v4).

Full inputs logits/targets [4,3,128,128,128] f32 are sharded along depth
D=128 into 8 slices of 16, converted to fp8-e4m3 on the host (targets {0,1}
exact; logits 3.6% rms rounding washes out over 25M-element sums, biasing
the loss ~1e-4 relative — far inside the 2e-2 gate), and packed in an
AUGMENTED layout: each 128-column chunk carries 4 extra columns
[1, 0, 0, 0].  A diag-trick matmul whose rhs is an augmented chunk then
produces, in PSUM column 128, the column-sums of its lhsT operand for free:

  xt bank (lhsT=t, rhs=x_aug):   diag = x*t,     col128 = sum(t)   per slab
  tp bank (lhsT=pred, rhs=t_aug): diag = t*pred,  col128 = sum(pred) per slab
  st bank (lhsT=s, rhs=t_aug):    diag = s*t (global)

which eliminates all ones-row matmuls.  All PE operands are fp8 -> double
pumped (2 cols/cycle).  ScalarE runs dense over the augmented tiles (the
deterministic contribution of the [1,0,0,0] columns to the sigmoid/ln
accumulators is subtracted exactly on the host).

Math (s := sigmoid(-x)):
  sum(prob) = N - sum(s);  sum(prob*t) = sum(t) - sum(s*t)
  bce_sum   = -sum(ln s) - sum(x*t);   pred = (x >= 0.5)
  ln uses bias 1e-5: guards ln(0) when fp8 underflows s for x > 6.9.

Input DMA rides a SINGLE queue: the DMA engines round-robin fairly across
outstanding transfers, so one queue = strict arrival order at ~400 GB/s,
while multiple queues delay the critical first chunks.  Order: x slabs
first (ScalarE is the critical path), t interleaved just-in-time for PE.

Engine budget per core: ScalarE ~46us (2 activation passes, 2 table loads)
= critical path; TensorE ~34us (3 fp8 diag quantities); VectorE ~31us
(pred + per-slab PSUM extractions); DMA in 6.5MB ~16us.

Device outputs per core, one stats tile [128, 64] f32 (+ small late tile):
  cols 0-4   sigmoid accums (5 instrs: slabs [0],[1],[2-3],[4-7],[8-11])
  cols 5-7   ln accums ([0-3],[4-7],[8-11])  (in stats2 [128,3])
  col  8     sum(s*t) global (masked diag reduce)
  cols 9-20  sum(t*pred) per slab      cols 21-32 sum(x*t) per slab
  cols 33-44 sum(pred) per slab        cols 45-56 sum(t) per slab
"""

import sys

if "/opt/trn_rl_repo" not in sys.path:
    sys.path.insert(0, "/opt/trn_rl_repo")

import numpy as np

import concourse.bacc as bacc
import concourse.mybir as mybir
from concourse import tile
from concourse.alu_op_type import AluOpType
from concourse.bass_utils import run_bass_kernel_spmd

# Problem geometry (hardcoded per harness contract).
B, C, D, H, W = 4, 3, 128, 128, 128
N_CORES = 8
D_SHARD = D // N_CORES            # 16
SLABS = B * C                     # 12 (b,c) slabs per core
P = 128
F = D_SHARD * H * W // P          # 2048 real cols per slab
N_TOTAL = B * C * D * H * W
NCH = F // 128                    # 16 chunks per slab
AUG = 132                         # 128 real + [1,0,0,0]
SF = NCH * AUG                    # 2112 aug cols per slab
TF = SLABS * SF                   # 25344 aug cols total

_CACHED = {}


def _build():
    if "nc" in _CACHED:
        return _CACHED["nc"]
    AFT = mybir.ActivationFunctionType
    f32 = mybir.dt.float32
    fp8 = mybir.dt.float8e4

    nc = bacc.Bacc("TRN2", target_bir_lowering=False, debug=False,
                   num_devices=N_CORES)
    x_d = nc.dram_tensor("logits", [P, TF], fp8, kind="ExternalInput")
    t_d = nc.dram_tensor("targets", [P, TF], fp8, kind="ExternalInput")
    id_d = nc.dram_tensor("ident", [P, AUG], fp8, kind="ExternalInput")
    st_d = nc.dram_tensor("stats", [P, 64], f32, kind="ExternalOutput")
    s2_d = nc.dram_tensor("stats2", [P, 3], f32, kind="ExternalOutput")

    SIG_SPLIT = [(0, 1), (1, 2), (2, 4), (4, 8), (8, 12)]
    XCH = [(0, 1), (1, 2), (2, 4), (4, 6), (6, 8), (8, 10), (10, 12)]
    TCH = [(0, 3), (3, 6), (6, 9), (9, 12)]
    # Single-queue arrival order: x leads, t just-in-time.
    ORDER = [("x", 0), ("x", 1), ("x", 2), ("t", 0), ("x", 3), ("x", 4),
             ("t", 1), ("x", 5), ("x", 6), ("t", 2), ("t", 3)]

    with tile.TileContext(nc) as tc:
        with (
            tc.tile_pool(name="data", bufs=1) as data_pool,
            tc.tile_pool(name="misc", bufs=1) as misc_pool,
            tc.tile_pool(name="psum", bufs=1, space="PSUM") as psum_pool,
        ):
            stats = misc_pool.tile([P, 64], f32)
            nc.vector.memset(stats[:], 0.0)
            stats2 = misc_pool.tile([P, 3], f32)
            nc.vector.memset(stats2[:], 0.0)
            lnbias = misc_pool.tile([P, 1], f32)
            nc.vector.memset(lnbias[:], 1e-5)
            ident = misc_pool.tile([P, AUG], fp8)

            NC = SLABS * NCH          # 192 chunks total
            xa = data_pool.tile([P, NC, AUG], fp8, name="xa")
            ta = data_pool.tile([P, NC, AUG], fp8, name="ta")
            s8 = data_pool.tile([P, NC, AUG], fp8, name="s8")
            pr = data_pool.tile([P, NC, AUG], fp8, name="pr")
            lo = data_pool.tile([P, NC, AUG], fp8, name="lo")

            # ---- Input DMA: one queue, strict order.
            first = True
            for kind, i in ORDER:
                if kind == "x":
                    a, b = XCH[i]
                    nc.sync.dma_start(xa[:, a * NCH:b * NCH, :],
                                      x_d[:, a * SF:b * SF])
                    if first:
                        nc.sync.dma_start(ident[:], id_d[:])
                        first = False
                else:
                    a, b = TCH[i]
                    nc.sync.dma_start(ta[:, a * NCH:b * NCH, :],
                                      t_d[:, a * SF:b * SF])

            # ---- ScalarE: dummy sigmoid first so the auto-inserted
            # ACT_TABLE_LOAD runs during the DMA wait; then the sigmoid
            # chain, one table switch, and a single ln pass.  Both real
            # passes use strided APs covering only the 128 real columns of
            # each 132-column chunk, so no aug-column corrections exist.
            dummy = misc_pool.tile([P, 1], fp8)
            nc.scalar.activation(dummy[:], lnbias[:, 0:1], AFT.Sigmoid)
            for a, b in SIG_SPLIT:
                nc.scalar.activation(s8[:, a * NCH:b * NCH, 0:128],
                                     xa[:, a * NCH:b * NCH, 0:128],
                                     AFT.Sigmoid, scale=-1.0)
            nc.scalar.activation(lo[:, :, 0:128], s8[:, :, 0:128], AFT.Ln,
                                 bias=lnbias[:, 0:1],
                                 accum_out=stats2[:, 0:1])

            # ---- VectorE: pred (dense over aug tiles, fp8 2x).
            for a, b in [(0, 2), (2, 4), (4, 6), (6, 8), (8, 10), (10, 12)]:
                nc.vector.tensor_scalar(out=pr[:, a * NCH:b * NCH, :],
                                        in0=xa[:, a * NCH:b * NCH, :],
                                        scalar1=0.5, scalar2=None,
                                        op0=AluOpType.is_ge)

            # ---- PSUM banks: st global + rotating xt/tp.
            p_st = psum_pool.tile([P, AUG], f32, name="p_st", tag="p_st")
            p_xt = [psum_pool.tile([P, AUG], f32, name=f"p_xt{i}", tag=f"p_xt{i}")
                    for i in range(2)]
            p_tp = [psum_pool.tile([P, AUG], f32, name=f"p_tp{i}", tag=f"p_tp{i}")
                    for i in range(2)]

            # ---- TensorE + extractions per slab.
            for s in range(SLABS):
                xt_b = p_xt[s % 2]
                tp_b = p_tp[s % 2]
                for c in range(NCH):
                    k = s * NCH + c
                    nc.tensor.matmul(xt_b[:, :], ta[:, k, 0:128],
                                     xa[:, k, :],
                                     start=(c == 0), stop=(c == NCH - 1))
                for c in range(NCH):
                    k = s * NCH + c
                    nc.tensor.matmul(tp_b[:, :], pr[:, k, 0:128],
                                     ta[:, k, :],
                                     start=(c == 0), stop=(c == NCH - 1))
                for c in range(NCH):
                    k = s * NCH + c
                    nc.tensor.matmul(p_st[:, :], s8[:, k, 0:128],
                                     ta[:, k, :],
                                     start=(s == 0 and c == 0),
                                     stop=(s == SLABS - 1 and c == NCH - 1))

                mx = misc_pool.tile([P, AUG], f32, name=f"mx{s}", tag="mx",
                                    bufs=2)
                nc.vector.tensor_tensor(out=mx[:], in0=xt_b[:, :],
                                        in1=ident[:], op=AluOpType.mult)
                nc.vector.tensor_scalar(out=mx[:], in0=mx[:], scalar1=1.0,
                                        scalar2=0.0, op0=AluOpType.mult,
                                        op1=AluOpType.add,
                                        accum_out=stats[:, 21 + s:22 + s])
                nc.vector.tensor_copy(stats[:, 45 + s:46 + s],
                                      xt_b[:, 128:129])
                mt = misc_pool.tile([P, AUG], f32, name=f"mt{s}", tag="mt",
                                    bufs=2)
                nc.vector.tensor_tensor(out=mt[:], in0=tp_b[:, :],
                                        in1=ident[:], op=AluOpType.mult)
                nc.vector.tensor_scalar(out=mt[:], in0=mt[:], scalar1=1.0,
                                        scalar2=0.0, op0=AluOpType.mult,
                                        op1=AluOpType.add,
                                        accum_out=stats[:, 9 + s:10 + s])
                nc.vector.tensor_copy(stats[:, 33 + s:34 + s],
                                      tp_b[:, 128:129])

            # ---- st global extraction (+ sum(s) from its ones-column).
            ms = misc_pool.tile([P, AUG], f32)
            nc.vector.tensor_tensor(out=ms[:], in0=p_st[:, :], in1=ident[:],
                                    op=AluOpType.mult)
            nc.vector.tensor_scalar(out=ms[:], in0=ms[:], scalar1=1.0,
                                    scalar2=0.0, op0=AluOpType.mult,
                                    op1=AluOpType.add,
                                    accum_out=stats[:, 8:9])
            nc.vector.tensor_copy(stats[:, 0:1], p_st[:, 128:129])
            nc.sync.dma_start(st_d[:], stats[:])
            nc.sync.dma_start(s2_d[:], stats2[:])

    nc.compile()
    _CACHED["nc"] = nc
    return nc


def _pack_aug(a):
    """[12, P, F] fp8 -> [P, TF] (partition-major) with [1,0,0,0] after
    each 128 cols, giving long contiguous DMA rows."""
    import ml_dtypes

    f8 = ml_dtypes.float8_e4m3
    n = a.reshape(SLABS, P, NCH, 128)
    out = np.zeros((SLABS, P, NCH, AUG), dtype=f8)
    out[..., :128] = n
    out[..., 128] = f8(1.0)
    return np.ascontiguousarray(
        out.transpose(1, 0, 2, 3).reshape(P, TF))


def _shard_inputs(logits: np.ndarray, targets: np.ndarray):
    import ml_dtypes

    f8 = ml_dtypes.float8_e4m3
    xb = np.ascontiguousarray(logits, dtype=np.float32).astype(f8)
    tb = np.ascontiguousarray(targets, dtype=np.float32).astype(f8)
    eye = np.zeros((P, AUG), dtype=np.float32)
    eye[:, :128] = np.eye(P, 128, dtype=np.float32)
    eye = eye.astype(f8)
    in_maps = []
    for i in range(N_CORES):
        sl = slice(i * D_SHARD, (i + 1) * D_SHARD)
        x = np.ascontiguousarray(xb[:, :, sl]).reshape(SLABS, P, F)
        t = np.ascontiguousarray(tb[:, :, sl]).reshape(SLABS, P, F)
        in_maps.append({"logits": _pack_aug(x), "targets": _pack_aug(t),
                        "ident": eye})
    return in_maps


def _combine(results):
    """Host-side reduction of per-core partials to the scalar loss."""
    EPS = 1e-9
    S_s = 0.0
    S_l = 0.0
    S_xt = 0.0
    S_st = 0.0
    S_tp = np.zeros(SLABS)
    S_t = np.zeros(SLABS)
    S_pred = np.zeros(SLABS)
    for r in results:
        st = r["stats"].astype(np.float64)
        s2 = r["stats2"].astype(np.float64)
        S_s += st[:, 0].sum()
        S_l += s2[:, 0].sum()
        S_st += st[:, 8].sum()
        S_tp += st[:, 9:21].sum(axis=0)
        S_xt += st[:, 21:33].sum()
        S_pred += st[:, 33:45].sum(axis=0)
        S_t += st[:, 45:57].sum(axis=0)

    sum_prob = N_TOTAL - S_s
    sum_pt = S_t.sum() - S_st                 # sum(prob * t)
    sum_sp = -S_l                             # sum(softplus(x))
    bce = (sum_sp - S_xt) / N_TOTAL

    union = sum_prob + S_t.sum()
    inter = 2.0 * sum_pt
    dice_loss = 1.0 - (inter + EPS) / union

    score = np.where(
        (S_t == 0) & (S_pred == 0),
        np.ones_like(S_t),
        (2.0 * S_tp + EPS) / (S_t + S_pred),
    ).reshape(B, C)
    per_class = score.mean(axis=0)

    loss = (bce + dice_loss * 0.5 + per_class[0] * 0.2
            + per_class[1] * 0.1 + per_class[2] * 0.2)
    return np.float32(loss)


def kernel(logits: np.ndarray, targets: np.ndarray) -> np.ndarray:
    nc = _build()
    in_maps = _shard_inputs(np.asarray(logits), np.asarray(targets))
    res = run_bass_kernel_spmd(nc, in_maps, list(range(N_CORES)))
    return _combine(res.results)


# revision 23
# speedup vs baseline: 1.2742x; 1.0083x over previous
"""BCE + weighted Dice loss on 8 Trainium2 NeuronCores (fp8, v4).

Full inputs logits/targets [4,3,128,128,128] f32 are sharded along depth
D=128 into 8 slices of 16, converted to fp8-e4m3 on the host (targets {0,1}
exact; logits 3.6% rms rounding washes out over 25M-element sums, biasing
the loss ~1e-4 relative — far inside the 2e-2 gate), and packed in an
AUGMENTED layout: each 128-column chunk carries 4 extra columns
[1, 0, 0, 0].  A diag-trick matmul whose rhs is an augmented chunk then
produces, in PSUM column 128, the column-sums of its lhsT operand for free:

  xt bank (lhsT=t, rhs=x_aug):   diag = x*t,     col128 = sum(t)   per slab
  tp bank (lhsT=pred, rhs=t_aug): diag = t*pred,  col128 = sum(pred) per slab
  st bank (lhsT=s, rhs=t_aug):    diag = s*t (global)

which eliminates all ones-row matmuls.  All PE operands are fp8 -> double
pumped (2 cols/cycle).  ScalarE runs dense over the augmented tiles (the
deterministic contribution of the [1,0,0,0] columns to the sigmoid/ln
accumulators is subtracted exactly on the host).

Math (s := sigmoid(-x)):
  sum(prob) = N - sum(s);  sum(prob*t) = sum(t) - sum(s*t)
  bce_sum   = -sum(ln s) - sum(x*t);   pred = (x >= 0.5)
  ln uses bias 1e-5: guards ln(0) when fp8 underflows s for x > 6.9.

Input DMA rides a SINGLE queue: the DMA engines round-robin fairly across
outstanding transfers, so one queue = strict arrival order at ~400 GB/s,
while multiple queues delay the critical first chunks.  Order: x slabs
first (ScalarE is the critical path), t interleaved just-in-time for PE.

Engine budget per core: ScalarE ~46us (2 activation passes, 2 table loads)
= critical path; TensorE ~34us (3 fp8 diag quantities); VectorE ~31us
(pred + per-slab PSUM extractions); DMA in 6.5MB ~16us.

Device outputs per core, one stats tile [128, 64] f32 (+ small late tile):
  cols 0-4   sigmoid accums (5 instrs: slabs [0],[1],[2-3],[4-7],[8-11])
  cols 5-7   ln accums ([0-3],[4-7],[8-11])  (in stats2 [128,3])
  col  8     sum(s*t) global (masked diag reduce)
  cols 9-20  sum(t*pred) per slab      cols 21-32 sum(x*t) per slab
  cols 33-44 sum(pred) per slab        cols 45-56 sum(t) per slab
"""

import sys

if "/opt/trn_rl_repo" not in sys.path:
    sys.path.insert(0, "/opt/trn_rl_repo")

import numpy as np

import concourse.bacc as bacc
import concourse.mybir as mybir
from concourse import tile
from concourse.alu_op_type import AluOpType
from concourse.bass_utils import run_bass_kernel_spmd

# Problem geometry (hardcoded per harness contract).
B, C, D, H, W = 4, 3, 128, 128, 128
N_CORES = 8
D_SHARD = D // N_CORES            # 16
SLABS = B * C                     # 12 (b,c) slabs per core
P = 128
F = D_SHARD * H * W // P          # 2048 real cols per slab
N_TOTAL = B * C * D * H * W
NCH = F // 128                    # 16 chunks per slab
AUG = 132                         # 128 real + [1,0,0,0]
SF = NCH * AUG                    # 2112 aug cols per slab
TF = SLABS * SF                   # 25344 aug cols total

_CACHED = {}


def _build():
    if "nc" in _CACHED:
        return _CACHED["nc"]
    AFT = mybir.ActivationFunctionType
    f32 = mybir.dt.float32
    fp8 = mybir.dt.float8e4

    nc = bacc.Bacc("TRN2", target_bir_lowering=False, debug=False,
                   num_devices=N_CORES)
    xh_d = nc.dram_tensor("logits_head", [2, P, SF], fp8,
                          kind="ExternalInput")
    xr_d = nc.dram_tensor("logits_rest", [5, P, 2 * SF], fp8,
                          kind="ExternalInput")
    t_d = nc.dram_tensor("targets", [4, P, 3 * SF], fp8,
                         kind="ExternalInput")
    id_d = nc.dram_tensor("ident", [P, AUG], fp8, kind="ExternalInput")
    st_d = nc.dram_tensor("stats", [P, 64], f32, kind="ExternalOutput")
    s2_d = nc.dram_tensor("stats2", [P, 3], f32, kind="ExternalOutput")

    SIG_SPLIT = [(0, 1), (1, 2), (2, 4), (4, 8), (8, 12)]
    XCH = [(0, 1), (1, 2), (2, 4), (4, 6), (6, 8), (8, 10), (10, 12)]
    TCH = [(0, 3), (3, 6), (6, 9), (9, 12)]
    # Single-queue arrival order: x leads, t just-in-time.
    ORDER = [("x", 0), ("x", 1), ("x", 2), ("t", 0), ("x", 3), ("x", 4),
             ("t", 1), ("x", 5), ("x", 6), ("t", 2), ("t", 3)]

    def x_src(i):
        # chunks 0,1 live in logits_head; 2.. in logits_rest
        return xh_d[i] if i < 2 else xr_d[i - 2]

    with tile.TileContext(nc) as tc:
        with (
            tc.tile_pool(name="data", bufs=1) as data_pool,
            tc.tile_pool(name="misc", bufs=1) as misc_pool,
            tc.tile_pool(name="psum", bufs=1, space="PSUM") as psum_pool,
        ):
            stats = misc_pool.tile([P, 64], f32)
            nc.vector.memset(stats[:], 0.0)
            stats2 = misc_pool.tile([P, 3], f32)
            nc.vector.memset(stats2[:], 0.0)
            lnbias = misc_pool.tile([P, 1], f32)
            nc.vector.memset(lnbias[:], 1e-5)
            ident = misc_pool.tile([P, AUG], fp8)

            NC = SLABS * NCH          # 192 chunks total
            xa = data_pool.tile([P, NC, AUG], fp8, name="xa")
            ta = data_pool.tile([P, NC, AUG], fp8, name="ta")
            s8 = data_pool.tile([P, NC, AUG], fp8, name="s8")
            pr = data_pool.tile([P, NC, AUG], fp8, name="pr")
            lo = data_pool.tile([P, NC, AUG], fp8, name="lo")

            # ---- Input DMA: one queue, strict order.
            nc.gpsimd.dma_start(ident[:], id_d[:])
            for kind, i in ORDER:
                if kind == "x":
                    a, b = XCH[i]
                    nc.sync.dma_start(xa[:, a * NCH:b * NCH, :], x_src(i))
                else:
                    a, b = TCH[i]
                    nc.sync.dma_start(ta[:, a * NCH:b * NCH, :], t_d[i])

            # ---- ScalarE: dummy sigmoid first so the auto-inserted
            # ACT_TABLE_LOAD runs during the DMA wait; then the sigmoid
            # chain, one table switch, and a single ln pass.  Both real
            # passes use strided APs covering only the 128 real columns of
            # each 132-column chunk, so no aug-column corrections exist.
            dummy = misc_pool.tile([P, 1], fp8)
            nc.scalar.activation(dummy[:], lnbias[:, 0:1], AFT.Sigmoid)
            for a, b in SIG_SPLIT:
                nc.scalar.activation(s8[:, a * NCH:b * NCH, 0:128],
                                     xa[:, a * NCH:b * NCH, 0:128],
                                     AFT.Sigmoid, scale=-1.0)
            nc.scalar.activation(lo[:, :, 0:128], s8[:, :, 0:128], AFT.Ln,
                                 bias=lnbias[:, 0:1],
                                 accum_out=stats2[:, 0:1])

            # ---- VectorE: pred (dense over aug tiles, fp8 2x).
            for a, b in [(0, 2), (2, 4), (4, 6), (6, 8), (8, 10), (10, 12)]:
                nc.vector.tensor_scalar(out=pr[:, a * NCH:b * NCH, :],
                                        in0=xa[:, a * NCH:b * NCH, :],
                                        scalar1=0.5, scalar2=None,
                                        op0=AluOpType.is_ge)

            # ---- PSUM banks: st global + rotating xt/tp.
            p_st = psum_pool.tile([P, AUG], f32, name="p_st", tag="p_st")
            p_xt = [psum_pool.tile([P, AUG], f32, name=f"p_xt{i}", tag=f"p_xt{i}")
                    for i in range(2)]
            p_tp = [psum_pool.tile([P, AUG], f32, name=f"p_tp{i}", tag=f"p_tp{i}")
                    for i in range(2)]

            # ---- TensorE + extractions per slab.
            for s in range(SLABS):
                xt_b = p_xt[s % 2]
                tp_b = p_tp[s % 2]
                for c in range(NCH):
                    k = s * NCH + c
                    nc.tensor.matmul(xt_b[:, :], ta[:, k, 0:128],
                                     xa[:, k, :],
                                     start=(c == 0), stop=(c == NCH - 1))
                for c in range(NCH):
                    k = s * NCH + c
                    nc.tensor.matmul(tp_b[:, :], pr[:, k, 0:128],
                                     ta[:, k, :],
                                     start=(c == 0), stop=(c == NCH - 1))
                for c in range(NCH):
                    k = s * NCH + c
                    nc.tensor.matmul(p_st[:, :], s8[:, k, 0:128],
                                     ta[:, k, :],
                                     start=(s == 0 and c == 0),
                                     stop=(s == SLABS - 1 and c == NCH - 1))

                mx = misc_pool.tile([P, AUG], f32, name=f"mx{s}", tag="mx",
                                    bufs=2)
                nc.vector.tensor_tensor(out=mx[:], in0=xt_b[:, :],
                                        in1=ident[:], op=AluOpType.mult)
                nc.vector.tensor_scalar(out=mx[:], in0=mx[:], scalar1=1.0,
                                        scalar2=0.0, op0=AluOpType.mult,
                                        op1=AluOpType.add,
                                        accum_out=stats[:, 21 + s:22 + s])
                nc.vector.tensor_copy(stats[:, 45 + s:46 + s],
                                      xt_b[:, 128:129])
                mt = misc_pool.tile([P, AUG], f32, name=f"mt{s}", tag="mt",
                                    bufs=2)
                nc.vector.tensor_tensor(out=mt[:], in0=tp_b[:, :],
                                        in1=ident[:], op=AluOpType.mult)
                nc.vector.tensor_scalar(out=mt[:], in0=mt[:], scalar1=1.0,
                                        scalar2=0.0, op0=AluOpType.mult,
                                        op1=AluOpType.add,
                                        accum_out=stats[:, 9 + s:10 + s])
                nc.vector.tensor_copy(stats[:, 33 + s:34 + s],
                                      tp_b[:, 128:129])

            # ---- st global extraction (+ sum(s) from its ones-column).
            ms = misc_pool.tile([P, AUG], f32)
            nc.vector.tensor_tensor(out=ms[:], in0=p_st[:, :], in1=ident[:],
                                    op=AluOpType.mult)
            nc.vector.tensor_scalar(out=ms[:], in0=ms[:], scalar1=1.0,
                                    scalar2=0.0, op0=AluOpType.mult,
                                    op1=AluOpType.add,
                                    accum_out=stats[:, 8:9])
            nc.vector.tensor_copy(stats[:, 0:1], p_st[:, 128:129])
            nc.sync.dma_start(st_d[:], stats[:])
            nc.sync.dma_start(s2_d[:], stats2[:])

    nc.compile()
    _CACHED["nc"] = nc
    return nc


def _pack_aug(a):
    """[12, P, F] fp8 -> [P, TF] (partition-major) with [1,0,0,0] after
    each 128 cols."""
    import ml_dtypes

    f8 = ml_dtypes.float8_e4m3
    n = a.reshape(SLABS, P, NCH, 128)
    out = np.zeros((SLABS, P, NCH, AUG), dtype=f8)
    out[..., :128] = n
    out[..., 128] = f8(1.0)
    return np.ascontiguousarray(
        out.transpose(1, 0, 2, 3).reshape(P, TF))


def _chunk(aug, ranges, width):
    """[P, TF] -> [n, P, width] stacking contiguous slab-range chunks."""
    return np.stack([np.ascontiguousarray(aug[:, a * SF:b * SF])
                     for a, b in ranges]).reshape(len(ranges), P, width)


def _shard_inputs(logits: np.ndarray, targets: np.ndarray):
    import ml_dtypes

    f8 = ml_dtypes.float8_e4m3
    xb = np.ascontiguousarray(logits, dtype=np.float32).astype(f8)
    tb = np.ascontiguousarray(targets, dtype=np.float32).astype(f8)
    eye = np.zeros((P, AUG), dtype=np.float32)
    eye[:, :128] = np.eye(P, 128, dtype=np.float32)
    eye = eye.astype(f8)
    in_maps = []
    for i in range(N_CORES):
        sl = slice(i * D_SHARD, (i + 1) * D_SHARD)
        x = np.ascontiguousarray(xb[:, :, sl]).reshape(SLABS, P, F)
        t = np.ascontiguousarray(tb[:, :, sl]).reshape(SLABS, P, F)
        xaug = _pack_aug(x)
        taug = _pack_aug(t)
        in_maps.append({
            "logits_head": _chunk(xaug, [(0, 1), (1, 2)], SF),
            "logits_rest": _chunk(xaug, [(2, 4), (4, 6), (6, 8), (8, 10),
                                         (10, 12)], 2 * SF),
            "targets": _chunk(taug, [(0, 3), (3, 6), (6, 9), (9, 12)],
                              3 * SF),
            "ident": eye,
        })
    return in_maps


def _combine(results):
    """Host-side reduction of per-core partials to the scalar loss."""
    EPS = 1e-9
    S_s = 0.0
    S_l = 0.0
    S_xt = 0.0
    S_st = 0.0
    S_tp = np.zeros(SLABS)
    S_t = np.zeros(SLABS)
    S_pred = np.zeros(SLABS)
    for r in results:
        st = r["stats"].astype(np.float64)
        s2 = r["stats2"].astype(np.float64)
        S_s += st[:, 0].sum()
        S_l += s2[:, 0].sum()
        S_st += st[:, 8].sum()
        S_tp += st[:, 9:21].sum(axis=0)
        S_xt += st[:, 21:33].sum()
        S_pred += st[:, 33:45].sum(axis=0)
        S_t += st[:, 45:57].sum(axis=0)

    sum_prob = N_TOTAL - S_s
    sum_pt = S_t.sum() - S_st                 # sum(prob * t)
    sum_sp = -S_l                             # sum(softplus(x))
    bce = (sum_sp - S_xt) / N_TOTAL

    union = sum_prob + S_t.sum()
    inter = 2.0 * sum_pt
    dice_loss = 1.0 - (inter + EPS) / union

    score = np.where(
        (S_t == 0) & (S_pred == 0),
        np.ones_like(S_t),
        (2.0 * S_tp + EPS) / (S_t + S_pred),
    ).reshape(B, C)
    per_class = score.mean(axis=0)

    loss = (bce + dice_loss * 0.5 + per_class[0] * 0.2
            + per_class[1] * 0.1 + per_class[2] * 0.2)
    return np.float32(loss)


def kernel(logits: np.ndarray, targets: np.ndarray) -> np.ndarray:
    nc = _build()
    in_maps = _shard_inputs(np.asarray(logits), np.asarray(targets))
    res = run_bass_kernel_spmd(nc, in_maps, list(range(N_CORES)))
    return _combine(res.results)
